# revision 8
# baseline (speedup 1.0000x reference)
"""Additive (Bahdanau) attention on 8 Trainium2 NeuronCores.

reference math (per batch b):
    qp = query @ Wq                          [Q, H]
    kp = key @ Wk                            [K, H]
    scores[q,k] = sum_h v[h] * tanh(qp[q,h] + kp[k,h])
    attention = softmax(scores, axis=k)      [Q, K]
    context = attention @ value              [Q, VD]
    returns (context, attention)

Sharding: fully data-parallel, core c handles batch b = c//2 and query rows
qh*256..qh*256+256 (qh = c%2). Softmax is over K which is kept whole per
core, so no collectives are needed.

Per-core device algorithm (h=128 lives on partitions):
  kpT [h,1024k] and qpT [h,256q] via PE matmuls (weights are lhsT as stored).
  For each q: ACT computes tanh(kpT + qpT[:,q]) in one instruction
  (per-partition bias) emitting bf16; PE reduces over h with v as the
  moving operand and the tanh tile as the stationary operand, writing
  column q of scoresT [k,q] PSUM tiles.  scores are bounded by sum|v| (~9)
  so softmax needs no max-subtraction.  exp on ACT (same table set as
  tanh), PE transposes to [q,k], second exp pass with accum_out gives the
  row sums, DVE reciprocal + per-partition scales produce attention and
  the context epilogue.
"""

import os
import sys

import numpy as np

for p in ("/opt/trn_rl_repo",):
    if p not in sys.path and os.path.isdir(p):
        sys.path.insert(0, p)

B, Q, K, QD, KD, VD, H = 4, 512, 1024, 512, 512, 512, 128
NCORES = 8
QS = Q // 2  # query rows per core

_NC_CACHE = None


def _build_nc(reps=1):
    from contextlib import ExitStack

    import concourse.bass as bass  # noqa: F401
    import concourse.tile as tile
    from concourse import bacc, mybir
    from concourse.masks import make_identity

    f32 = mybir.dt.float32
    bf16 = mybir.dt.bfloat16
    AF = mybir.ActivationFunctionType

    nc = bacc.Bacc(
        "TRN2",
        target_bir_lowering=False,
        debug=False,
        enable_asserts=True,
        num_devices=NCORES,
    )

    qT = nc.declare_dram_parameter("qT", [QD, QS], f32, isOutput=False)
    kT = nc.declare_dram_parameter("kT", [KD, K], f32, isOutput=False)
    val = nc.declare_dram_parameter("val", [K, VD], f32, isOutput=False)
    wq = nc.declare_dram_parameter("wq", [QD, H], f32, isOutput=False)
    wk = nc.declare_dram_parameter("wk", [KD, H], f32, isOutput=False)
    vv = nc.declare_dram_parameter("vv", [H, 1], f32, isOutput=False)
    ctx_o = nc.declare_dram_parameter("ctx", [QS, VD], f32, isOutput=True)
    att_o = nc.declare_dram_parameter("att", [QS, K], f32, isOutput=True)

    DQ = QD // 128  # 4 contraction chunks for the projections
    KC = K // 128  # 8 key chunks

    with tile.TileContext(nc) as tc, ExitStack() as ctx:
        if reps > 1:
            # wall-clock benchmarking only: repeat the whole body on-device
            ctx.enter_context(tc.For_i(0, reps, 1))
        const = ctx.enter_context(tc.tile_pool(name="const", bufs=1))

        kT_sb = const.tile([128, DQ, K], f32)
        qT_sb = const.tile([128, DQ, QS], f32)
        wk_sb = const.tile([128, DQ, H], f32)
        wq_sb = const.tile([128, DQ, H], f32)
        val_sb = const.tile([128, KC, VD], f32)
        val_bf = const.tile([128, KC, VD], bf16)
        v_sb = const.tile([128, 1], f32)
        v_bf = const.tile([128, 1], bf16)
        ident = const.tile([128, 128], f32)
        kpT_sb = const.tile([128, K], f32)
        qpT_sb = const.tile([128, QS], f32)

        for i in range(DQ):
            nc.sync.dma_start(out=kT_sb[:, i, :], in_=kT[128 * i : 128 * (i + 1), :])
            nc.sync.dma_start(out=qT_sb[:, i, :], in_=qT[128 * i : 128 * (i + 1), :])
            nc.sync.dma_start(out=wk_sb[:, i, :], in_=wk[128 * i : 128 * (i + 1), :])
            nc.sync.dma_start(out=wq_sb[:, i, :], in_=wq[128 * i : 128 * (i + 1), :])
        for i in range(KC):
            nc.sync.dma_start(out=val_sb[:, i, :], in_=val[128 * i : 128 * (i + 1), :])
        nc.sync.dma_start(out=v_sb[:], in_=vv[:])

        make_identity(nc, ident[:])
        nc.vector.tensor_copy(out=v_bf[:], in_=v_sb[:])
        for i in range(KC):
            nc.vector.tensor_copy(out=val_bf[:, i, :], in_=val_sb[:, i, :])

        # ---- projections: kpT [h, K], qpT [h, QS] ----
        with tc.tile_pool(name="proj_psum", bufs=2, space="PSUM") as proj_psum:
            for half in range(2):
                pt = proj_psum.tile([128, 512], f32)
                for d in range(DQ):
                    nc.tensor.matmul(
                        pt[:],
                        wk_sb[:, d, :],
                        kT_sb[:, d, 512 * half : 512 * (half + 1)],
                        start=(d == 0),
                        stop=(d == DQ - 1),
                    )
                nc.vector.tensor_copy(
                    out=kpT_sb[:, 512 * half : 512 * (half + 1)], in_=pt[:]
                )
            pt = proj_psum.tile([128, QS], f32)
            for d in range(DQ):
                nc.tensor.matmul(
                    pt[:],
                    wq_sb[:, d, :],
                    qT_sb[:, d, :],
                    start=(d == 0),
                    stop=(d == DQ - 1),
                )
            nc.vector.tensor_copy(out=qpT_sb[:], in_=pt[:])

        expT_bf = const.tile([128, KC, QS], bf16)
        sT_sb = const.tile([128, KC, QS], f32)

        # ---- main loop: tanh + h-reduction into scoresT [k, q] ----
        with (
            tc.tile_pool(name="scores", bufs=1, space="PSUM") as scores_pool,
            tc.tile_pool(name="tanh", bufs=3) as tanh_pool,
        ):
            sc = [
                scores_pool.tile([128, 2, QS], f32, name=f"sc{i}", tag=f"sc{i}")
                for i in range(KC // 2)
            ]
            for q in range(QS):
                tq = tanh_pool.tile([128, K], bf16)
                nc.scalar.activation(
                    tq[:], kpT_sb[:], AF.Tanh, bias=qpT_sb[:, q : q + 1]
                )
                for c in range(KC):
                    nc.tensor.matmul(
                        sc[c // 2][:, c % 2, q : q + 1],
                        tq[:, 128 * c : 128 * (c + 1)],
                        v_bf[:],
                        start=True,
                        stop=True,
                    )

            # exp for the context matmul (bf16) + fp32 copy for transposing
            for c in range(KC):
                nc.scalar.activation(
                    expT_bf[:, c, :], sc[c // 2][:, c % 2, :], AF.Exp
                )
                nc.vector.tensor_copy(out=sT_sb[:, c, :], in_=sc[c // 2][:, c % 2, :])

        # ---- per q-block: transpose, softmax, outputs ----
        with (
            tc.tile_pool(name="s_psum", bufs=2, space="PSUM") as s_pool,
            tc.tile_pool(name="ctx_psum", bufs=2, space="PSUM") as ctx_pool,
            tc.tile_pool(name="e_sb", bufs=2) as e_pool,
            tc.tile_pool(name="small", bufs=8) as small_pool,
            tc.tile_pool(name="outs", bufs=4) as out_pool,
        ):
            for qb in range(QS // 128):
                s_ps = s_pool.tile([128, KC, 128], f32)
                for c in range(KC):
                    nc.tensor.transpose(
                        s_ps[:, c, :],
                        sT_sb[:, c, 128 * qb : 128 * (qb + 1)],
                        ident[:],
                    )
                e_sb = e_pool.tile([128, K], f32)
                sums = small_pool.tile([128, 1], f32)
                nc.scalar.activation(
                    e_sb[:], s_ps[:, :, :], AF.Exp, accum_out=sums[:]
                )
                r = small_pool.tile([128, 1], f32)
                nc.vector.reciprocal(r[:], sums[:])

                att_sb = out_pool.tile([128, K], f32)
                nc.vector.tensor_scalar_mul(att_sb[:], e_sb[:], r[:])
                nc.sync.dma_start(
                    out=att_o[128 * qb : 128 * (qb + 1), :], in_=att_sb[:]
                )

                cps = ctx_pool.tile([128, VD], f32)
                for c in range(KC):
                    nc.tensor.matmul(
                        cps[:],
                        expT_bf[:, c, 128 * qb : 128 * (qb + 1)],
                        val_bf[:, c, :],
                        start=(c == 0),
                        stop=(c == KC - 1),
                    )
                ctx_sb = out_pool.tile([128, VD], f32)
                nc.vector.tensor_scalar_mul(ctx_sb[:], cps[:], r[:])
                nc.sync.dma_start(
                    out=ctx_o[128 * qb : 128 * (qb + 1), :], in_=ctx_sb[:]
                )

    nc.compile()
    return nc


def get_nc(reps=1):
    global _NC_CACHE
    if reps != 1:
        return _build_nc(reps=reps)
    if _NC_CACHE is None:
        _NC_CACHE = _build_nc()
    return _NC_CACHE


def make_in_maps(query, key, value, Wq, Wk, v):
    query = np.asarray(query, dtype=np.float32)
    key = np.asarray(key, dtype=np.float32)
    value = np.asarray(value, dtype=np.float32)
    Wq = np.ascontiguousarray(np.asarray(Wq, dtype=np.float32))
    Wk = np.ascontiguousarray(np.asarray(Wk, dtype=np.float32))
    vv = np.ascontiguousarray(np.asarray(v, dtype=np.float32).reshape(H, 1))

    in_maps = []
    for c in range(NCORES):
        b, qh = divmod(c, 2)
        in_maps.append(
            {
                "qT": np.ascontiguousarray(query[b, qh * QS : (qh + 1) * QS, :].T),
                "kT": np.ascontiguousarray(key[b].T),
                "val": np.ascontiguousarray(value[b]),
                "wq": Wq,
                "wk": Wk,
                "vv": vv,
            }
        )
    return in_maps


def assemble(results):
    context = np.empty((B, Q, VD), np.float32)
    attention = np.empty((B, Q, K), np.float32)
    for c in range(NCORES):
        b, qh = divmod(c, 2)
        context[b, qh * QS : (qh + 1) * QS, :] = results[c]["ctx"]
        attention[b, qh * QS : (qh + 1) * QS, :] = results[c]["att"]
    return context, attention


def kernel(query, key, value, Wq, Wk, v):
    from concourse.bass_utils import run_bass_kernel_spmd

    nc = get_nc()
    in_maps = make_in_maps(query, key, value, Wq, Wk, v)
    res = run_bass_kernel_spmd(nc, in_maps, core_ids=list(range(NCORES))).results
    return assemble(res)


# revision 39
# speedup vs baseline: 1.2205x; 1.2205x over previous
"""Additive (Bahdanau) attention on 8 Trainium2 NeuronCores.

reference math (per batch b):
    qp = query @ Wq                          [Q, H]
    kp = key @ Wk                            [K, H]
    scores[q,k] = sum_h v[h] * tanh(qp[q,h] + kp[k,h])
    attention = softmax(scores, axis=k)      [Q, K]
    context = attention @ value              [Q, VD]
    returns (context, attention)

Sharding: fully data-parallel, core c handles batch b = c//2 and query rows
qh*256..qh*256+256 (qh = c%2). Softmax is over K which is kept whole per
core, so no collectives are needed.

Per-core device algorithm (h=128 lives on partitions):
  Matrix inputs are cast to bf16 on the host (halves the DMA bytes,
  full-rate PE; the projections accumulate in fp32 PSUM so the tanh
  inputs stay accurate).  DVE pre-adds qp[q] (per-partition scalar) onto
  kpT into [128, 8x1024] fp32 sum tiles; one big ACT tanh per 8 queries
  emits bf16 (ACT is the bottleneck engine: Q*K*H/128 lanes / 1.2GHz =
  218us floor).  PE reduces over h with v as the bf16 moving operand and
  each tanh tile as the stationary operand, writing column q of scoresT
  [k,q] PSUM tiles.  scores are bounded by sum|v| (~9) so softmax needs
  no max-subtraction.  exp on ACT (tanh and exp share one table set ->
  no table switches), PE transposes scoresT to [q,k], a second exp pass
  with accum_out yields the row sums, DVE reciprocal + per-partition
  scales produce attention; the bf16 context matmul runs on PE straight
  from the exp and value tiles.
"""

import os
import sys

import numpy as np

for p in ("/opt/trn_rl_repo",):
    if p not in sys.path and os.path.isdir(p):
        sys.path.insert(0, p)

B, Q, K, QD, KD, VD, H = 4, 512, 1024, 512, 512, 512, 128
NCORES = 8
QS = Q // 2  # query rows per core

_NC_CACHE = None


def _build_nc(reps=1):
    from contextlib import ExitStack

    import concourse.tile as tile
    from concourse import bacc, mybir
    from concourse.masks import make_identity

    f32 = mybir.dt.float32
    bf16 = mybir.dt.bfloat16
    AF = mybir.ActivationFunctionType

    nc = bacc.Bacc(
        "TRN2",
        target_bir_lowering=False,
        debug=False,
        enable_asserts=True,
        num_devices=NCORES,
    )

    qT = nc.declare_dram_parameter("qT", [QD, QS], bf16, isOutput=False)
    kT = nc.declare_dram_parameter("kT", [KD, K], bf16, isOutput=False)
    val = nc.declare_dram_parameter("val", [K, VD], bf16, isOutput=False)
    wq = nc.declare_dram_parameter("wq", [QD, H], bf16, isOutput=False)
    wk = nc.declare_dram_parameter("wk", [KD, H], bf16, isOutput=False)
    vv = nc.declare_dram_parameter("vv", [H, 1], bf16, isOutput=False)
    ctx_o = nc.declare_dram_parameter("ctx", [QS, VD], f32, isOutput=True)
    att_o = nc.declare_dram_parameter("att", [QS, K], f32, isOutput=True)

    DQ = QD // 128  # 4 contraction chunks for the projections
    KC = K // 128  # 8 key chunks

    with tile.TileContext(nc) as tc, ExitStack() as ctx:
        if reps > 1:
            # wall-clock benchmarking only: repeat the whole body on-device
            ctx.enter_context(tc.For_i(0, reps, 1))
        const = ctx.enter_context(tc.tile_pool(name="const", bufs=1))

        wk_sb = const.tile([128, DQ, H], bf16)
        wq_sb = const.tile([128, DQ, H], bf16)
        v_bf = const.tile([128, 1], bf16)
        ident = const.tile([128, 128], f32)
        kpT_sb = const.tile([128, K], f32)
        qpT_sb = const.tile([128, QS], f32)
        expT_bf = const.tile([128, KC, QS], bf16)
        sT_sb = const.tile([128, KC, QS], f32)

        # preload the exp_and_others ACT table at t~0 (no data deps) so the
        # first real tanh doesn't pay the table-load on the critical path
        scratch = const.tile([128, 1], f32)
        nc.vector.memset(scratch[:], 0.0)
        nc.scalar.activation(scratch[:], scratch[:], AF.Tanh)

        # ---- staging pools. `staging` (kT/qT) is released right after the
        # projections so the main-loop pools' SBUF only waits on that;
        # `valstage` sits lower on the stack and releases later, so the val
        # load stays off the critical path (val is consumed directly as
        # float32r by the context matmul -- no cast needed). ----
        valstage = ctx.enter_context(tc.tile_pool(name="valstage", bufs=1))
        val_sb = valstage.tile([128, KC, VD], bf16)
        with tc.tile_pool(name="staging", bufs=1) as staging:
            kT_sb = staging.tile([128, DQ, K], bf16)
            qT_sb = staging.tile([128, DQ, QS], bf16)
            # Split input DMAs over the SP and ACT HWDGE queues so the
            # kT/wk bytes that gate the first projection land as early as
            # possible; v is tiny and first (its cast heads the strict-FIFO
            # DVE stream).
            nc.sync.dma_start(out=v_bf[:], in_=vv[:])
            for half in range(2):
                for i in range(DQ):
                    eng = nc.sync if i % 2 == 0 else nc.scalar
                    if half == 0:
                        eng.dma_start(
                            out=wk_sb[:, i, :], in_=wk[128 * i : 128 * (i + 1), :]
                        )
                    eng.dma_start(
                        out=kT_sb[:, i, 512 * half : 512 * (half + 1)],
                        in_=kT[128 * i : 128 * (i + 1), 512 * half : 512 * (half + 1)],
                    )
            # POOL queue: wq/qT, then val
            for i in range(DQ):
                nc.gpsimd.dma_start(
                    out=wq_sb[:, i, :], in_=wq[128 * i : 128 * (i + 1), :]
                )
                nc.gpsimd.dma_start(
                    out=qT_sb[:, i, :], in_=qT[128 * i : 128 * (i + 1), :]
                )
            for i in range(KC):
                nc.gpsimd.dma_start(
                    out=val_sb[:, i, :], in_=val[128 * i : 128 * (i + 1), :]
                )
            make_identity(nc, ident[:])

            # ---- projections: kpT [h, K], qpT [h, QS] (bf16 inputs from the
            # host: full-rate PE, half the DMA bytes; fp32 PSUM accumulate) ----
            # PSUM->SBUF copies go on ACT (idle during warmup; keeps the DVE
            # stream free for the first adds). Order kp-h0, qp, kp-h1: the
            # first group's adds need kp-h0 + qp first.
            with tc.tile_pool(name="proj_psum", bufs=3, space="PSUM") as proj_psum:
                pt0 = proj_psum.tile([128, 512], f32, name="pt0", tag="pt")
                for d in range(DQ):
                    nc.tensor.matmul(
                        pt0[:],
                        wk_sb[:, d, :],
                        kT_sb[:, d, 0:512],
                        start=(d == 0),
                        stop=(d == DQ - 1),
                    )
                nc.scalar.copy(out=kpT_sb[:, 0:512], in_=pt0[:])

                ptq = proj_psum.tile([128, QS], f32, name="ptq", tag="pt")
                for d in range(DQ):
                    nc.tensor.matmul(
                        ptq[:],
                        wq_sb[:, d, :],
                        qT_sb[:, d, :],
                        start=(d == 0),
                        stop=(d == DQ - 1),
                    )
                nc.scalar.copy(out=qpT_sb[:], in_=ptq[:])

                pt1 = proj_psum.tile([128, 512], f32, name="pt1", tag="pt")
                for d in range(DQ):
                    nc.tensor.matmul(
                        pt1[:],
                        wk_sb[:, d, :],
                        kT_sb[:, d, 512:1024],
                        start=(d == 0),
                        stop=(d == DQ - 1),
                    )
                nc.scalar.copy(out=kpT_sb[:, 512:1024], in_=pt1[:])

        # ---- main loop: add + tanh + h-reduction into scoresT [k, q],
        #      epilogue for each 128-query block interleaved after its
        #      columns complete ----
        GQ = 8  # queries per tanh batch (amortizes ACT per-instr overhead)
        with (
            tc.tile_pool(name="scores", bufs=1, space="PSUM") as scores_pool,
            tc.tile_pool(name="sums", bufs=2) as sum_pool,
            tc.tile_pool(name="tanh", bufs=2) as tanh_pool,
            tc.tile_pool(name="s_psum", bufs=1, space="PSUM") as s_pool,
            tc.tile_pool(name="ctx_psum", bufs=2, space="PSUM") as ctx_pool,
            tc.tile_pool(name="e_sb", bufs=2) as e_pool,
            tc.tile_pool(name="small", bufs=8) as small_pool,
            tc.tile_pool(name="outs", bufs=4) as out_pool,
        ):
            # scoresT per q-block: 2 bank-tiles of 4 chunks x 128 cols
            sc = [
                [
                    scores_pool.tile(
                        [128, 4, 128], f32, name=f"sc{qb}_{i}", tag=f"sc{qb}_{i}"
                    )
                    for i in range(2)
                ]
                for qb in range(2)
            ]

            def epilogue(qb):
                # scoresT chunk tiles for this q-block -> exp on ACT (fp32,
                # feeds the f32r context matmul) + fp32 copy -> PE transpose
                # -> second exp with row-sum accumulation -> 1/sum scales ->
                # DMA out
                for c in range(KC):
                    nc.scalar.activation(
                        expT_bf[:, c, 128 * qb : 128 * (qb + 1)],
                        sc[qb][c // 4][:, c % 4, :],
                        AF.Exp,
                    )
                    nc.vector.tensor_copy(
                        out=sT_sb[:, c, 128 * qb : 128 * (qb + 1)],
                        in_=sc[qb][c // 4][:, c % 4, :],
                    )
                # context matmul needs only expT/val (bf16): runs on PE
                # while ACT does the second exp pass
                cps = ctx_pool.tile([128, VD], f32, name="cps", tag="cps")
                for c in range(KC):
                    nc.tensor.matmul(
                        cps[:],
                        expT_bf[:, c, 128 * qb : 128 * (qb + 1)],
                        val_sb[:, c, :],
                        start=(c == 0),
                        stop=(c == KC - 1),
                    )
                s_ps = s_pool.tile([128, KC, 128], f32, name="s_ps", tag="s_ps")
                for c in range(KC):
                    nc.tensor.transpose(
                        s_ps[:, c, :],
                        sT_sb[:, c, 128 * qb : 128 * (qb + 1)],
                        ident[:],
                    )
                e_sb = e_pool.tile([128, K], f32, name="e_sb", tag="e_sb")
                sums = small_pool.tile([128, 1], f32, name="sums", tag="sums")
                nc.scalar.activation(
                    e_sb[:], s_ps[:, :, :], AF.Exp, accum_out=sums[:]
                )
                r = small_pool.tile([128, 1], f32, name="r", tag="r")
                nc.vector.reciprocal(r[:], sums[:])

                att_sb = out_pool.tile([128, K], f32, name="att_sb", tag="att_sb")
                nc.vector.tensor_scalar_mul(att_sb[:], e_sb[:], r[:])
                nc.sync.dma_start(
                    out=att_o[128 * qb : 128 * (qb + 1), :], in_=att_sb[:]
                )
                ctx_sb = out_pool.tile([128, VD], f32, name="ctx_sb", tag="ctx_sb")
                nc.vector.tensor_scalar_mul(ctx_sb[:], cps[:], r[:])
                nc.sync.dma_start(
                    out=ctx_o[128 * qb : 128 * (qb + 1), :], in_=ctx_sb[:]
                )

            groups_per_block = 128 // GQ
            for g in range(QS // GQ):
                sum_t = sum_pool.tile([128, GQ, K], f32, name="sum_t", tag="sum_t")
                tq = tanh_pool.tile([128, GQ, K], bf16, name="tq", tag="tq")
                if g == 0:
                    # ramp-up: half-granularity adds (start on kpT half 0 as
                    # soon as it's copied) and two half-size tanhs so ACT
                    # starts earlier
                    for jh in range(2):
                        for j in range(GQ // 2 * jh, GQ // 2 * (jh + 1)):
                            for half in range(2):
                                nc.vector.tensor_scalar_add(
                                    sum_t[:, j, 512 * half : 512 * (half + 1)],
                                    kpT_sb[:, 512 * half : 512 * (half + 1)],
                                    qpT_sb[:, j : j + 1],
                                )
                        nc.scalar.activation(
                            tq[:, GQ // 2 * jh : GQ // 2 * (jh + 1), :],
                            sum_t[:, GQ // 2 * jh : GQ // 2 * (jh + 1), :],
                            AF.Tanh,
                        )
                else:
                    for j in range(GQ):
                        q = GQ * g + j
                        nc.vector.tensor_scalar_add(
                            sum_t[:, j, :], kpT_sb[:], qpT_sb[:, q : q + 1]
                        )
                    nc.scalar.activation(tq[:, :, :], sum_t[:, :, :], AF.Tanh)
                for j in range(GQ):
                    q = GQ * g + j
                    qb, qo = divmod(q, 128)
                    for c in range(KC):
                        nc.tensor.matmul(
                            sc[qb][c // 4][:, c % 4, qo : qo + 1],
                            tq[:, j, 128 * c : 128 * (c + 1)],
                            v_bf[:],
                            start=True,
                            stop=True,
                        )
                if (g + 1) % groups_per_block == 0:
                    epilogue((g + 1) // groups_per_block - 1)

    nc.compile()
    return nc


def get_nc(reps=1):
    global _NC_CACHE
    if reps != 1:
        return _build_nc(reps=reps)
    if _NC_CACHE is None:
        _NC_CACHE = _build_nc()
    return _NC_CACHE


def make_in_maps(query, key, value, Wq, Wk, v):
    import ml_dtypes

    bf = ml_dtypes.bfloat16
    query = np.asarray(query, dtype=np.float32)
    key = np.asarray(key, dtype=np.float32)
    value = np.asarray(value, dtype=np.float32)
    Wq = np.ascontiguousarray(np.asarray(Wq, dtype=np.float32).astype(bf))
    Wk = np.ascontiguousarray(np.asarray(Wk, dtype=np.float32).astype(bf))
    vv = np.ascontiguousarray(
        np.asarray(v, dtype=np.float32).reshape(H, 1).astype(bf)
    )

    in_maps = []
    for c in range(NCORES):
        b, qh = divmod(c, 2)
        in_maps.append(
            {
                "qT": np.ascontiguousarray(
                    query[b, qh * QS : (qh + 1) * QS, :].T.astype(bf)
                ),
                "kT": np.ascontiguousarray(key[b].T.astype(bf)),
                "val": np.ascontiguousarray(value[b].astype(bf)),
                "wq": Wq,
                "wk": Wk,
                "vv": vv,
            }
        )
    return in_maps


def assemble(results):
    context = np.empty((B, Q, VD), np.float32)
    attention = np.empty((B, Q, K), np.float32)
    for c in range(NCORES):
        b, qh = divmod(c, 2)
        context[b, qh * QS : (qh + 1) * QS, :] = results[c]["ctx"]
        attention[b, qh * QS : (qh + 1) * QS, :] = results[c]["att"]
    return context, attention


def kernel(query, key, value, Wq, Wk, v):
    from concourse.bass_utils import run_bass_kernel_spmd

    nc = get_nc()
    in_maps = make_in_maps(query, key, value, Wq, Wk, v)
    res = run_bass_kernel_spmd(nc, in_maps, core_ids=list(range(NCORES))).results
    return assemble(res)
